# revision 1
# baseline (speedup 1.0000x reference)
"""Gated Linear Attention on 8 Trainium2 NeuronCores.

Sharding: one (batch, head) pair per core (B=2 x H=4 = 8 cores). The recurrent
state is independent per (batch, head); each core computes its head's full
pipeline (projections -> chunked GLA scan -> RMS-norm scale -> silu gate ->
output projection) and emits a partial [N, D] output; the host sums the 4 head
partials per batch.

Device algorithm (chunked, chunk C=128, all f32):
  g'' = min(softplus(-(x@Wz + bgk2)), 48)        (= -16*log-decay, >= 0)
  b'' = global running cumsum of g'' over time (per feature)
  E = exp(-b''/16), En = exp(+b''/16)
  q~ = q*E, k~ = k*En   (global-decay scaling; exp args bounded ~47 for this
                         data distribution, safe in f32)
  intra: AT[s,t] = (k~ q~^T)[s,t] masked s<=t ;  o^T = v^T @ AT + W^T q~^T
  state: W += k~^T v    (accumulates in PSUM across chunks, no rescaling)
  out_partial = rms_r * ((o^T * silu-gate^T)^T @ (rms_w*Wout_head))

Host folds Wgk1@Wgk2 -> Wz and rms_w into Wout; x is fed pre-transposed.
"""

import os
from contextlib import ExitStack

import numpy as np

import concourse.bass as bass
import concourse.tile as tile
from concourse import bacc, mybir
from concourse.tile_rust import add_dep_helper
from concourse.bass_utils import run_bass_kernel_spmd

F32 = mybir.dt.float32
AF = mybir.ActivationFunctionType

B, N, D, H = 2, 1024, 1024, 4
KD, VD, DK, DV = 512, 1024, 128, 256
C = 128                    # chunk length (= token partitions)
NCH = N // C               # 8 chunks
NK = D // 128              # 8 contraction tiles
BLOBW = 896                # blob cols: q128 | k128 | v256 | z128 | gate256
EPS = 1e-5

# module-level stash so test.py can grab profiling results
LAST_RESULTS = None


def _emit_kernel(ctx: ExitStack, tc: "tile.TileContext", ap: dict):
    nc = tc.nc

    # Chain all PE instructions in program order. PE executes in-order anyway,
    # but the Tile scheduler may otherwise reorder range-disjoint matmuls
    # within a PSUM bank, breaking has_written clear ordering (start=True
    # clears the whole 2KB zero region).
    pe_prev = [None]

    def mm(*args, **kw):
        inst = nc.tensor.matmul(*args, **kw)
        if pe_prev[0] is not None:
            add_dep_helper(inst.ins, pe_prev[0], sync=False, reason="pe-order")
        pe_prev[0] = inst.ins
        return inst

    def tr_(out, in_, ident):
        inst = nc.tensor.transpose(out, in_, ident)
        if pe_prev[0] is not None:
            add_dep_helper(inst.ins, pe_prev[0], sync=False, reason="pe-order")
        pe_prev[0] = inst.ins
        return inst
    xT, wblob, woutT = ap["xT"], ap["wblob"], ap["woutT"]
    bgk2, lmask, ident, out = ap["bgk2"], ap["lmask"], ap["ident"], ap["out"]

    consts = ctx.enter_context(tc.tile_pool(name="consts", bufs=1))
    wpool = ctx.enter_context(tc.tile_pool(name="wpool", bufs=1))
    work = ctx.enter_context(tc.tile_pool(name="work", bufs=2))
    wide = ctx.enter_context(tc.tile_pool(name="wide", bufs=2))
    outp = ctx.enter_context(tc.tile_pool(name="outp", bufs=3))
    wst = ctx.enter_context(tc.tile_pool(name="wst", bufs=2))
    psum = ctx.enter_context(tc.tile_pool(name="psum", bufs=1, space="PSUM"))

    # ---- constants ----
    L_sb = consts.tile([128, 128], F32)          # L[s,t]=1 iff s<=t (triu)
    nc.sync.dma_start(out=L_sb[:], in_=lmask[:])
    id_sb = consts.tile([128, 128], F32)
    nc.sync.dma_start(out=id_sb[:], in_=ident[:])
    bg_sb = consts.tile([1, 128], F32)
    nc.sync.dma_start(out=bg_sb[:], in_=bgk2[:])
    ones_col = consts.tile([128, 1], F32)
    nc.vector.memset(ones_col[:], 1.0)
    ones_row = consts.tile([1, 128], F32)
    nc.vector.memset(ones_row[:], 1.0)
    w0_sb = consts.tile([128, DV], F32)          # zero state for chunk 0
    nc.vector.memset(w0_sb[:], 0.0)
    boff0 = consts.tile([1, 128], F32)
    nc.vector.memset(boff0[:], 0.0)
    eps_sb = consts.tile([128, 1], F32)
    nc.vector.memset(eps_sb[:], EPS)

    # ---- weights + x: interleave so chunk-0 compute can start early ----
    wsb = wpool.tile([128, NK, BLOBW], F32)
    xsb = wpool.tile([128, NK, N], F32)
    for k in range(NK):
        nc.sync.dma_start(out=wsb[:, k, :], in_=wblob[k])
        nc.sync.dma_start(out=xsb[:, k, 0:C], in_=xT[k, :, 0:C])
    wout_sb = wpool.tile([128, 2, D], F32)
    for j in range(2):
        nc.sync.dma_start(out=wout_sb[:, j, :], in_=woutT[j])
    for c in range(1, NCH):
        for k in range(NK):
            nc.sync.dma_start(out=xsb[:, k, c * C:(c + 1) * C],
                              in_=xT[k, :, c * C:(c + 1) * C])

    # persistent PSUM bank: cols 0:256 = state W accumulator, cols 256:384 =
    # cumsum carry accumulator (row 0 only). All matmuls into this bank use
    # skip_group_check (single never-closed accumulation group).
    w_ps = psum.tile([128, 512], F32, tag="wps")

    w_prev = w0_sb        # SBUF copy of state before current chunk
    boff_prev = boff0     # [1,128] cumsum carry

    for c in range(NCH):
        tok = slice(c * C, (c + 1) * C)

        # ---------- projections (x stationary), + bias + cumsum in bank1 ----
        # proj psum [128,1024] = 2 banks:
        #   bank0 cols 0:512   = q(0:128) k(128:256) v(256:512)  token-major
        #   bank1 cols 512:1024= z(512:640) gate(640:896) b(896:1024)
        proj = psum.tile([128, 1024], F32, tag="proj")
        for k in range(NK):
            lhs = xsb[:, k, tok]
            mm(proj[:, 0:512], lhsT=lhs, rhs=wsb[:, k, 0:512],
                             start=(k == 0), stop=(k == NK - 1))
            mm(proj[:, 512:896], lhsT=lhs, rhs=wsb[:, k, 512:896],
                             start=(k == 0), stop=False)
        # z += bgk2 (K=1 rank-1 matmul; closes the bank1 group so z/gate
        # become readable; the later b matmuls continue writing this bank with
        # skip_group_check)
        bias_mm = mm(proj[:, 512:640], lhsT=ones_row[:], rhs=bg_sb[:],
                         start=False, stop=True)

        # g'' = min(softplus(-z), 48); softplus(-z) = ln(1 + exp(-z)).
        # (Only exp/ln/square/copy ACT funcs are used in this kernel so the
        # whole run stays on one ACT table set - table reloads cost ~1.3us.)
        e1 = work.tile([128, 128], F32, tag="e1")
        nc.scalar.activation(e1[:], proj[:, 512:640], AF.Exp, scale=-1.0)
        u1 = work.tile([128, 128], F32, tag="u1")
        nc.vector.tensor_scalar_add(u1[:], e1[:], 1.0)
        sp = work.tile([128, 128], F32, tag="sp")
        nc.scalar.activation(sp[:], u1[:], AF.Ln)
        g_tm = work.tile([128, 128], F32, tag="g")
        nc.vector.tensor_scalar_min(g_tm[:], sp[:], 48.0)

        # b'' = L^T @ g'' + carry  (still bank1 group; L-matmul overwrites
        # because its elements' has_written bits were cleared at k==0 start)
        mm(proj[:, 896:1024], lhsT=L_sb[:], rhs=g_tm[:],
                         start=False, stop=False, skip_group_check=True)
        mm(proj[:, 896:1024], lhsT=ones_row[:], rhs=boff_prev[:],
                         start=False, stop=False, skip_group_check=True)

        # E = exp(-b''/16), En = exp(+b''/16)
        E_sb = work.tile([128, 128], F32, tag="E")
        nc.scalar.activation(E_sb[:], proj[:, 896:1024], AF.Exp, scale=-1.0 / 16.0)
        En_sb = work.tile([128, 128], F32, tag="En")
        nc.scalar.activation(En_sb[:], proj[:, 896:1024], AF.Exp, scale=1.0 / 16.0)

        # q~ = q * E ; k~ = k * En  (fused psum eviction)
        qt_tm = work.tile([128, 128], F32, tag="qt")
        nc.vector.tensor_mul(qt_tm[:], proj[:, 0:128], E_sb[:])
        kt_tm = work.tile([128, 128], F32, tag="kt")
        nc.vector.tensor_mul(kt_tm[:], proj[:, 128:256], En_sb[:])
        v_tm = wide.tile([128, DV], F32, tag="v")
        nc.scalar.copy(v_tm[:], proj[:, 256:512])

        # gate = silu(u) = u * 1/(1+exp(-u)), token-major (exp + DVE ops so we
        # stay on the exp/ln ACT table set)
        eg = wide.tile([128, DV], F32, tag="eg")
        _i = nc.scalar.activation(eg[:], proj[:, 640:896], AF.Exp, scale=-1.0)
        add_dep_helper(_i.ins, bias_mm.ins, sync=False,
                       reason="read gate after bank1 group close")
        ug = wide.tile([128, DV], F32, tag="ug")
        nc.vector.tensor_scalar_add(ug[:], eg[:], 1.0)
        sg = wide.tile([128, DV], F32, tag="sg")
        nc.vector.reciprocal(sg[:], ug[:])
        gate_tm = wide.tile([128, DV], F32, tag="gate")
        _i = nc.vector.tensor_mul(gate_tm[:], proj[:, 640:896], sg[:])
        add_dep_helper(_i.ins, bias_mm.ins, sync=False,
                       reason="read gate after bank1 group close")

        # ---------- transposes (PE): q~, k~, gate halves -> [feat, t] -------
        tr = psum.tile([128, 512], F32, tag="tr")
        tr_(tr[:, 0:128], qt_tm[:], id_sb[:])
        qtT = work.tile([128, 128], F32, tag="qtT")
        nc.vector.tensor_copy(qtT[:], tr[:, 0:128])
        tr_(tr[:, 128:256], kt_tm[:], id_sb[:])
        ktT = work.tile([128, 128], F32, tag="ktT")
        nc.vector.tensor_copy(ktT[:], tr[:, 128:256])
        tr_(tr[:, 256:384], gate_tm[:, 0:128], id_sb[:])
        tr_(tr[:, 384:512], gate_tm[:, 128:256], id_sb[:])
        gateT = wide.tile([128, DV], F32, tag="gateT")
        nc.scalar.copy(gateT[:], tr[:, 256:512])

        # ---------- intra-chunk attention ----------------------------------
        at_ps = psum.tile([128, 128], F32, tag="at")
        mm(at_ps[:], lhsT=ktT[:], rhs=qtT[:], start=True, stop=True)
        at_m = work.tile([128, 128], F32, tag="atm")
        nc.vector.tensor_mul(at_m[:], at_ps[:], L_sb[:])   # mask s<=t

        # ---------- o^T = v^T AT + W_prev^T q~^T ; ssq ----------------------
        # ot psum [128, 257] one bank: cols 0:128 dv-half0, 128:256 dv-half1,
        # 256:257 ssq (one accumulation group, start on first, stop on last)
        ot = psum.tile([128, 257], F32, tag="ot")
        mm(ot[:, 0:128], lhsT=v_tm[:, 0:128], rhs=at_m[:],
                         start=True, stop=False)
        mm(ot[:, 0:128], lhsT=w_prev[:, 0:128], rhs=qtT[:],
                         start=False, stop=False)
        mm(ot[:, 128:256], lhsT=v_tm[:, 128:256], rhs=at_m[:],
                         start=False, stop=False)
        mm(ot[:, 128:256], lhsT=w_prev[:, 128:256], rhs=qtT[:],
                         start=False, stop=True)

        # squares for RMS (read o^T from psum)
        sq = wide.tile([128, DV], F32, tag="sq")
        nc.scalar.square(sq[:], ot[:, 0:256])
        mm(ot[:, 256:257], lhsT=sq[:, 0:128], rhs=ones_col[:],
                         start=False, stop=False, skip_group_check=True)
        mm(ot[:, 256:257], lhsT=sq[:, 128:256], rhs=ones_col[:],
                         start=False, stop=False, skip_group_check=True)

        # r = (ssq/DV + eps)^(-1/2) = exp(-0.5 * ln(ssq/DV + eps))
        s_sb = work.tile([128, 1], F32, tag="s")
        nc.scalar.activation(s_sb[:], ot[:, 256:257], AF.Ln,
                             scale=1.0 / DV, bias=eps_sb[:])
        r_sb = work.tile([128, 1], F32, tag="r")
        nc.scalar.activation(r_sb[:], s_sb[:], AF.Exp, scale=-0.5)

        # gated o^T
        og = wide.tile([128, DV], F32, tag="og")
        nc.vector.tensor_mul(og[:], ot[:, 0:256], gateT[:])

        # ---------- state update (PSUM accumulate), evict for next chunk ----
        if c < NCH - 1:
            # W += k~^T v. c==0's start=True marks the whole bank's zero
            # regions (all 128 partition rows) pending, which also sets up the
            # carry region for the colsum matmul below.
            mm(w_ps[:, 0:256], lhsT=kt_tm[:], rhs=v_tm[:],
                             start=(c == 0), stop=False, skip_group_check=True)
            # carry: boff += colsum(g'') (row 0 of the carry region)
            mm(w_ps[0:1, 256:384], lhsT=ones_col[:], rhs=g_tm[:],
                             start=False, stop=False, skip_group_check=True)
            w_new = wst.tile([128, DV], F32, tag="wsb")
            nc.scalar.copy(w_new[:], w_ps[:, 0:256])
            w_prev = w_new
            boff = work.tile([1, 128], F32, tag="boff")
            nc.vector.tensor_copy(boff[:], w_ps[0:1, 256:384])
            boff_prev = boff

        # ---------- final projection + deferred RMS scale -------------------
        fin = psum.tile([128, 1024], F32, tag="fin")
        for nb in range(2):
            cols = slice(nb * 512, (nb + 1) * 512)
            mm(fin[:, cols], lhsT=og[:, 0:128],
                             rhs=wout_sb[:, 0, cols], start=True, stop=False)
            mm(fin[:, cols], lhsT=og[:, 128:256],
                             rhs=wout_sb[:, 1, cols], start=False, stop=True)
            o_sb = outp.tile([128, 512], F32, tag="osb")
            nc.vector.tensor_scalar_mul(o_sb[:], fin[:, cols], r_sb[:])
            nc.sync.dma_start(out=out[tok, cols], in_=o_sb[:])


def _build_nc():
    nc = bacc.Bacc("TRN2", target_bir_lowering=False, debug=False, num_devices=8)
    ap = {
        "xT": nc.dram_tensor("xT", [NK, 128, N], F32, kind="ExternalInput").ap(),
        "wblob": nc.dram_tensor("wblob", [NK, 128, BLOBW], F32,
                                kind="ExternalInput").ap(),
        "woutT": nc.dram_tensor("woutT", [2, 128, D], F32,
                                kind="ExternalInput").ap(),
        "bgk2": nc.dram_tensor("bgk2", [1, 128], F32, kind="ExternalInput").ap(),
        "lmask": nc.dram_tensor("lmask", [128, 128], F32,
                                kind="ExternalInput").ap(),
        "ident": nc.dram_tensor("ident", [128, 128], F32,
                                kind="ExternalInput").ap(),
        "out": nc.dram_tensor("out", [N, D], F32, kind="ExternalOutput").ap(),
    }
    with tile.TileContext(nc) as tc:
        with ExitStack() as ctx:
            _emit_kernel(ctx, tc, ap)
    nc.compile()
    return nc


def kernel(x, Wq, Wk, Wv, Wg, Wgk1, Wgk2, bgk2, Wout, rms_w):
    global LAST_RESULTS
    x = np.asarray(x, np.float32)
    Wz = (np.asarray(Wgk1, np.float32) @ np.asarray(Wgk2, np.float32))
    L = np.triu(np.ones((C, C), np.float32))
    I128 = np.eye(128, dtype=np.float32)

    in_maps = []
    for core in range(8):
        b, h = core // H, core % H
        xTb = np.ascontiguousarray(x[b].T).reshape(NK, 128, N)
        blob = np.ascontiguousarray(np.concatenate([
            Wq[:, h * DK:(h + 1) * DK], Wk[:, h * DK:(h + 1) * DK],
            Wv[:, h * DV:(h + 1) * DV], Wz[:, h * DK:(h + 1) * DK],
            Wg[:, h * DV:(h + 1) * DV]], axis=1).astype(np.float32)
        ).reshape(NK, 128, BLOBW)
        woutP = np.ascontiguousarray(
            (np.asarray(rms_w, np.float32)[:, None]
             * np.asarray(Wout, np.float32)[h * DV:(h + 1) * DV])
        ).reshape(2, 128, D)
        in_maps.append({
            "xT": xTb,
            "wblob": blob,
            "woutT": woutP,
            "bgk2": np.ascontiguousarray(
                np.asarray(bgk2, np.float32)[h * DK:(h + 1) * DK][None, :]),
            "lmask": L,
            "ident": I128,
        })

    nc = _build_nc()
    trace = os.environ.get("BASSGLA_TRACE", "0") == "1"
    res = run_bass_kernel_spmd(nc, in_maps, list(range(8)), trace=trace)
    LAST_RESULTS = res

    out = np.zeros((B, N, D), np.float32)
    for core in range(8):
        out[core // H] += res.results[core]["out"]
    return out



# revision 5
# speedup vs baseline: 1.7435x; 1.7435x over previous
"""Gated Linear Attention on 8 Trainium2 NeuronCores.

Sharding: one (batch, head) pair per core (B=2 x H=4 = 8 cores). Each core
computes its head's full pipeline and emits a partial [N, D] output (bf16);
the host sums the 4 head partials per batch in f32.

v2 design (vs f32 baseline):
  * All heavy matmuls in bf16 (1 PE cycle/row vs 4 for fp32). PSUM accumulates
    f32. Cumsum matmul (L^T g) stays f32.
  * Per-chunk LOCAL decay instead of a global running cumsum: within chunk c,
    b = L^T g'' (local inclusive cumsum, no carry chain). Intra-chunk attention
    uses q~=q*exp(-b/16), k~=k*exp(+b/16). Cross-chunk state is rescaled once
    per chunk by the per-feature factor f = exp(-b_last/16):
        W_c = diag(f) * (W_{c-1} + k~^T v)
    applied for free as the per-partition `scale=` AP of the PSUM->SBUF copy.
    Local exponent args are <= ~6 (vs ~47 global), so bf16/f32 are safe.
  * ACT table discipline (table loads cost 1.3us each; baseline had 33):
    softplus (exp+ln) batched globally up front; the main loop uses only
    {exp, square, copy} (one table); RMS-rsqrt (ln+exp) and silu (tanh:
    silu(u) = 0.5u*(1+tanh(u/2)) - tanh shares exp's table) deferred to a
    final phase. 5 table loads total.
  * RMS scale r folded into the silu gate (token-major, per-partition scalar),
    so the final projection needs no extra scaling pass.
  * bf16 I/O: inputs pre-cast on host; output partials are bf16, summed f32
    on host.
"""

import os
from contextlib import ExitStack

import numpy as np
import ml_dtypes

import concourse.bass as bass
import concourse.tile as tile
from concourse import bacc, mybir
from concourse.tile_rust import add_dep_helper
from concourse.bass_utils import run_bass_kernel_spmd

F32 = mybir.dt.float32
BF16 = mybir.dt.bfloat16
AF = mybir.ActivationFunctionType
ALU = mybir.AluOpType

B, N, D, H = 2, 1024, 1024, 4
KD, VD, DK, DV = 512, 1024, 128, 256
C = 128                    # chunk length (= token partitions)
NCH = N // C               # 8 chunks
NK = D // 128              # 8 contraction tiles
BLOBW = 768                # blob cols: q128 | k128 | v256 | gate256
GLN = 16.0
EPS = 1e-5

# module-level stash so test.py can grab profiling results
LAST_RESULTS = None


def _emit_kernel(ctx: ExitStack, tc: "tile.TileContext", ap: dict):
    nc = tc.nc

    # Chain all PE instructions in program order (PE executes in-order; this
    # keeps the Tile scheduler from reordering matmuls within a PSUM bank,
    # which would break has_written clear ordering).
    pe_prev = [None]

    def mm(*args, **kw):
        inst = nc.tensor.matmul(*args, **kw)
        if pe_prev[0] is not None:
            add_dep_helper(inst.ins, pe_prev[0], sync=False, reason="pe-order")
        pe_prev[0] = inst.ins
        return inst

    def tr_(out, in_, ident):
        inst = nc.tensor.transpose(out, in_, ident)
        if pe_prev[0] is not None:
            add_dep_helper(inst.ins, pe_prev[0], sync=False, reason="pe-order")
        pe_prev[0] = inst.ins
        return inst

    xT, wz, wblob, woutT = ap["xT"], ap["wz"], ap["wblob"], ap["woutT"]
    bgk2, lmask, ident32, identb = ap["bgk2"], ap["lmask"], ap["ident32"], ap["identb"]
    out = ap["out"]

    consts = ctx.enter_context(tc.tile_pool(name="consts", bufs=1))
    wpool = ctx.enter_context(tc.tile_pool(name="wpool", bufs=1))
    gat = ctx.enter_context(tc.tile_pool(name="gat", bufs=1))
    work = ctx.enter_context(tc.tile_pool(name="work", bufs=2))
    wst = ctx.enter_context(tc.tile_pool(name="wst", bufs=2))
    store = ctx.enter_context(tc.tile_pool(name="store", bufs=1))
    outp = ctx.enter_context(tc.tile_pool(name="outp", bufs=3))
    ppool = ctx.enter_context(tc.tile_pool(name="ppool", bufs=2, space="PSUM"))
    ptr = ctx.enter_context(tc.tile_pool(name="ptr", bufs=2, space="PSUM"))
    pao = ctx.enter_context(tc.tile_pool(name="pao", bufs=1, space="PSUM"))
    pst = ctx.enter_context(tc.tile_pool(name="pst", bufs=1, space="PSUM"))

    # ---- constants ----
    L_sb = consts.tile([128, 128], F32)          # L[s,t]=1 iff s<=t (triu)
    nc.sync.dma_start(out=L_sb[:], in_=lmask[:])
    id_sb = consts.tile([128, 128], F32)
    nc.sync.dma_start(out=id_sb[:], in_=ident32[:])
    idb_sb = consts.tile([128, 128], BF16)
    nc.sync.dma_start(out=idb_sb[:], in_=identb[:])
    bg_sb = consts.tile([1, 128], BF16)
    nc.sync.dma_start(out=bg_sb[:], in_=bgk2[:])
    ones_row = consts.tile([1, 128], BF16)
    nc.vector.memset(ones_row[:], 1.0)
    ones_col = consts.tile([128, 1], BF16)
    nc.vector.memset(ones_col[:], 1.0)
    eps_sb = consts.tile([128, 1], F32)
    nc.vector.memset(eps_sb[:], EPS)

    # ---- weights + x (bf16) ----
    wz_sb = wpool.tile([128, NK, 128], BF16)
    for k in range(NK):
        nc.sync.dma_start(out=wz_sb[:, k, :], in_=wz[k])
    xsb = wpool.tile([128, NK, N], BF16)
    for c in range(NCH):
        for k in range(NK):
            nc.sync.dma_start(out=xsb[:, k, c * C:(c + 1) * C],
                              in_=xT[k, :, c * C:(c + 1) * C])
    wb_sb = wpool.tile([128, NK, BLOBW], BF16)
    for k in range(NK):
        nc.sync.dma_start(out=wb_sb[:, k, :], in_=wblob[k])
    wout_sb = wpool.tile([128, 2, D], BF16)
    for j in range(2):
        nc.sync.dma_start(out=wout_sb[:, j, :], in_=woutT[j])

    # ---- Phase A: z = x @ Wz + bgk2 for all chunks -> one [128,1024] psum ---
    zps = ppool.tile([128, 1024], F32, tag="proj")
    for c in range(NCH):
        zc = zps[:, c * C:(c + 1) * C]
        first_bank = c in (0, 4)
        last_bank = c in (3, 7)
        for k in range(NK):
            mm(zc, lhsT=xsb[:, k, c * C:(c + 1) * C], rhs=wz_sb[:, k, :],
               start=(first_bank and k == 0), stop=False)
        mm(zc, lhsT=ones_row[:], rhs=bg_sb[:], start=False, stop=last_bank)

    # ---- Phase B: g'' = min(softplus(-z), 48) for all chunks (batched) -----
    e1 = gat.tile([128, 1024], F32)
    u1 = gat.tile([128, 1024], F32)
    sp = gat.tile([128, 1024], F32)
    g_all = gat.tile([128, 1024], F32)
    for hb in range(2):
        cols = slice(hb * 512, (hb + 1) * 512)
        nc.scalar.activation(e1[:, cols], zps[:, cols], AF.Exp, scale=-1.0)
    for hb in range(2):
        cols = slice(hb * 512, (hb + 1) * 512)
        nc.vector.tensor_scalar_add(u1[:, cols], e1[:, cols], 1.0)
    for hb in range(2):
        cols = slice(hb * 512, (hb + 1) * 512)
        nc.scalar.activation(sp[:, cols], u1[:, cols], AF.Ln)
    for hb in range(2):
        cols = slice(hb * 512, (hb + 1) * 512)
        nc.vector.tensor_scalar_min(g_all[:, cols], sp[:, cols], 48.0)

    # ---- main loop ---------------------------------------------------------
    # Per chunk: proj psum [128,1024] = bank0 {q 0:128 | k 128:256 | v 256:512}
    # bank1 {gate 512:768 | b_loc 768:896}. The L-matmul (local cumsum) closes
    # bank1. Emission is software-pipelined: proj(c+1) is emitted before the
    # small ops of chunk c so the PE stays busy while ACT/DVE evict chunk c.
    def emit_proj(c):
        proj = ppool.tile([128, 1024], F32, tag="proj")
        tok = slice(c * C, (c + 1) * C)
        for k in range(NK):
            lhs = xsb[:, k, tok]
            mm(proj[:, 0:512], lhsT=lhs, rhs=wb_sb[:, k, 0:512],
               start=(k == 0), stop=(k == NK - 1))
            mm(proj[:, 512:768], lhsT=lhs, rhs=wb_sb[:, k, 512:768],
               start=(k == 0), stop=False)
        # b_loc = L^T @ g_c (local inclusive cumsum; closes bank1)
        mm(proj[:, 768:896], lhsT=L_sb[:], rhs=g_all[:, tok],
           start=False, stop=True)
        return proj

    state = {"w_prev": None}

    def emit_smalls(c, proj):
        tok = slice(c * C, (c + 1) * C)
        # evictions (ACT) + decay factors
        b_sb = work.tile([128, 128], F32, tag="b")
        nc.scalar.copy(b_sb[:], proj[:, 768:896])
        tr = ptr.tile([128, 512], F32, tag="tr")
        tr_(tr[:, 0:128], b_sb[:], id_sb[:])          # bT [feat, tok]
        En_tok = work.tile([128, 128], F32, tag="Ent")
        nc.scalar.activation(En_tok[:], b_sb[:], AF.Exp, scale=1.0 / GLN)
        ET = work.tile([128, 128], F32, tag="ET")
        nc.scalar.activation(ET[:], tr[:, 0:128], AF.Exp, scale=-1.0 / GLN)
        EnT = work.tile([128, 128], F32, tag="EnT")
        nc.scalar.activation(EnT[:], tr[:, 0:128], AF.Exp, scale=1.0 / GLN)
        f_vec = work.tile([128, 1], F32, tag="f")     # exp(-b_last/16) per feat
        nc.scalar.activation(f_vec[:], tr[:, 127:128], AF.Exp, scale=-1.0 / GLN)

        q_sb = work.tile([128, 128], F32, tag="q")
        nc.scalar.copy(q_sb[:], proj[:, 0:128])
        tr_(tr[:, 128:256], q_sb[:], id_sb[:])
        qtT = work.tile([128, 128], BF16, tag="qtT")
        nc.vector.tensor_mul(qtT[:], tr[:, 128:256], ET[:])

        k_sb = work.tile([128, 128], F32, tag="k")
        nc.scalar.copy(k_sb[:], proj[:, 128:256])
        tr_(tr[:, 256:384], k_sb[:], id_sb[:])
        ktT = work.tile([128, 128], BF16, tag="ktT")
        nc.vector.tensor_mul(ktT[:], tr[:, 256:384], EnT[:])
        kt_tm = work.tile([128, 128], BF16, tag="kt")
        nc.vector.tensor_mul(kt_tm[:], k_sb[:], En_tok[:])

        v_tm = work.tile([128, DV], BF16, tag="v")
        nc.scalar.copy(v_tm[:], proj[:, 256:512])
        ug = store.tile([128, DV], F32, tag=f"ug{c}")
        nc.scalar.copy(ug[:], proj[:, 512:768])

        # intra-chunk attention: AT[s,t] masked s<=t
        ao = pao.tile([128, 512], F32, tag="ao")      # at 0:128 | oT 128:384
        mm(ao[:, 0:128], lhsT=ktT[:], rhs=qtT[:], start=True, stop=True)
        at_m = work.tile([128, 128], BF16, tag="atm")
        nc.vector.tensor_mul(at_m[:], ao[:, 0:128], L_sb[:])

        # oT = W_prev^T q~^T + v^T AT  (two dv halves)
        w_prev = state["w_prev"]
        if c > 0:
            mm(ao[:, 128:256], lhsT=w_prev[:, 0:128], rhs=qtT[:],
               start=False, stop=False, skip_group_check=True)
            mm(ao[:, 256:384], lhsT=w_prev[:, 128:256], rhs=qtT[:],
               start=False, stop=False, skip_group_check=True)
        mm(ao[:, 128:256], lhsT=v_tm[:, 0:128], rhs=at_m[:],
           start=False, stop=False, skip_group_check=True)
        mm(ao[:, 256:384], lhsT=v_tm[:, 128:256], rhs=at_m[:],
           start=False, stop=False, skip_group_check=True)

        # state: W_c = diag(f) (W_{c-1} + k~^T v)
        st = pst.tile([128, DV], F32, tag="st")
        mm(st[:], lhsT=kt_tm[:], rhs=v_tm[:], start=True, stop=(c == 0))
        if c > 0:
            mm(st[:], lhsT=idb_sb[:], rhs=w_prev[:], start=False, stop=True)
        w_new = wst.tile([128, DV], BF16, tag="w")
        nc.scalar.activation(w_new[:], st[:], AF.Copy, scale=f_vec[:])
        state["w_prev"] = w_new

        # ssq per token -> spare column of the at/ot bank, then to SBUF
        sq = work.tile([128, DV], BF16, tag="sq")
        nc.scalar.square(sq[:], ao[:, 128:384])
        mm(ao[:, 384:385], lhsT=sq[:, 0:128], rhs=ones_col[:],
           start=False, stop=False, skip_group_check=True)
        mm(ao[:, 384:385], lhsT=sq[:, 128:256], rhs=ones_col[:],
           start=False, stop=False, skip_group_check=True)
        nc.vector.tensor_copy(ssq_all[:, c:c + 1], ao[:, 384:385])

        oT = store.tile([128, DV], BF16, tag=f"oT{c}")
        nc.vector.tensor_copy(oT[:], ao[:, 128:384])
        state[f"oT{c}"] = oT
        state[f"ug{c}"] = ug

    ssq_all = gat.tile([128, 8], F32)

    prev_proj = emit_proj(0)
    for c in range(NCH):
        nxt = emit_proj(c + 1) if c + 1 < NCH else None
        emit_smalls(c, prev_proj)
        prev_proj = nxt

    # ---- Phase D: RMS scale, silu gate (via tanh), final projection --------
    s_sb = work.tile([128, 8], F32, tag="s")
    nc.scalar.activation(s_sb[:], ssq_all[:], AF.Ln, scale=1.0 / DV,
                         bias=eps_sb[:])
    r_all = work.tile([128, 8], F32, tag="r")
    nc.scalar.activation(r_all[:], s_sb[:], AF.Exp, scale=-0.5)

    for c in range(NCH):
        tok = slice(c * C, (c + 1) * C)
        ug = state[f"ug{c}"]
        oT = state[f"oT{c}"]
        th = work.tile([128, DV], F32, tag="th")
        nc.scalar.activation(th[:], ug[:], AF.Tanh, scale=0.5)
        thp = work.tile([128, DV], F32, tag="thp")
        nc.vector.tensor_scalar(thp[:], th[:], 0.5, 0.5, ALU.mult, ALU.add)
        # gate = silu(ug) * r = (ug*r) * (0.5 + 0.5*tanh(ug/2))
        gate_tm = work.tile([128, DV], F32, tag="gate")
        nc.vector.scalar_tensor_tensor(gate_tm[:], ug[:], r_all[:, c:c + 1],
                                       thp[:], ALU.mult, ALU.mult)
        tr2 = ptr.tile([128, 512], F32, tag="tr")
        tr_(tr2[:, 0:128], gate_tm[:, 0:128], id_sb[:])
        tr_(tr2[:, 128:256], gate_tm[:, 128:256], id_sb[:])
        gateT = work.tile([128, DV], BF16, tag="gT")
        nc.scalar.copy(gateT[:], tr2[:, 0:256])
        og = work.tile([128, DV], BF16, tag="og")
        nc.vector.tensor_mul(og[:], oT[:], gateT[:])

        fin = ppool.tile([128, 1024], F32, tag="proj")
        for nb in range(2):
            cols = slice(nb * 512, (nb + 1) * 512)
            mm(fin[:, cols], lhsT=og[:, 0:128],
               rhs=wout_sb[:, 0, cols], start=True, stop=False)
            mm(fin[:, cols], lhsT=og[:, 128:256],
               rhs=wout_sb[:, 1, cols], start=False, stop=True)
        o_sb0 = outp.tile([128, 512], BF16, tag="o0")
        nc.scalar.copy(o_sb0[:], fin[:, 0:512])
        nc.sync.dma_start(out=out[tok, 0:512], in_=o_sb0[:])
        o_sb1 = outp.tile([128, 512], BF16, tag="o1")
        nc.vector.tensor_copy(o_sb1[:], fin[:, 512:1024])
        nc.sync.dma_start(out=out[tok, 512:1024], in_=o_sb1[:])


def _build_nc():
    nc = bacc.Bacc("TRN2", target_bir_lowering=False, debug=False, num_devices=8)
    ap = {
        "xT": nc.dram_tensor("xT", [NK, 128, N], BF16, kind="ExternalInput").ap(),
        "wz": nc.dram_tensor("wz", [NK, 128, 128], BF16,
                             kind="ExternalInput").ap(),
        "wblob": nc.dram_tensor("wblob", [NK, 128, BLOBW], BF16,
                                kind="ExternalInput").ap(),
        "woutT": nc.dram_tensor("woutT", [2, 128, D], BF16,
                                kind="ExternalInput").ap(),
        "bgk2": nc.dram_tensor("bgk2", [1, 128], BF16, kind="ExternalInput").ap(),
        "lmask": nc.dram_tensor("lmask", [128, 128], F32,
                                kind="ExternalInput").ap(),
        "ident32": nc.dram_tensor("ident32", [128, 128], F32,
                                  kind="ExternalInput").ap(),
        "identb": nc.dram_tensor("identb", [128, 128], BF16,
                                 kind="ExternalInput").ap(),
        "out": nc.dram_tensor("out", [N, D], BF16, kind="ExternalOutput").ap(),
    }
    with tile.TileContext(nc) as tc:
        with ExitStack() as ctx:
            _emit_kernel(ctx, tc, ap)
    nc.compile()
    return nc


def kernel(x, Wq, Wk, Wv, Wg, Wgk1, Wgk2, bgk2, Wout, rms_w):
    global LAST_RESULTS
    BF = ml_dtypes.bfloat16
    x = np.asarray(x, np.float32)
    Wz = (np.asarray(Wgk1, np.float32) @ np.asarray(Wgk2, np.float32))
    L = np.triu(np.ones((C, C), np.float32))
    I32 = np.eye(128, dtype=np.float32)
    Ib = np.eye(128, dtype=np.float32).astype(BF)

    in_maps = []
    for core in range(8):
        b, h = core // H, core % H
        xTb = np.ascontiguousarray(x[b].T).reshape(NK, 128, N).astype(BF)
        wzb = np.ascontiguousarray(
            Wz[:, h * DK:(h + 1) * DK]).reshape(NK, 128, 128).astype(BF)
        blob = np.ascontiguousarray(np.concatenate([
            Wq[:, h * DK:(h + 1) * DK], Wk[:, h * DK:(h + 1) * DK],
            Wv[:, h * DV:(h + 1) * DV], Wg[:, h * DV:(h + 1) * DV]],
            axis=1).astype(np.float32)).reshape(NK, 128, BLOBW).astype(BF)
        woutP = np.ascontiguousarray(
            (np.asarray(rms_w, np.float32)[:, None]
             * np.asarray(Wout, np.float32)[h * DV:(h + 1) * DV])
        ).reshape(2, 128, D).astype(BF)
        in_maps.append({
            "xT": xTb,
            "wz": wzb,
            "wblob": blob,
            "woutT": woutP,
            "bgk2": np.ascontiguousarray(
                np.asarray(bgk2, np.float32)[h * DK:(h + 1) * DK][None, :]
            ).astype(BF),
            "lmask": L,
            "ident32": I32,
            "identb": Ib,
        })

    nc = _build_nc()
    trace = os.environ.get("BASSGLA_TRACE", "0") == "1"
    res = run_bass_kernel_spmd(nc, in_maps, list(range(8)), trace=trace)
    LAST_RESULTS = res

    out = np.zeros((B, N, D), np.float32)
    for core in range(8):
        out[core // H] += np.asarray(res.results[core]["out"], np.float32)
    return out


# revision 6
# speedup vs baseline: 2.2043x; 1.2642x over previous
"""Gated Linear Attention on 8 Trainium2 NeuronCores.

Sharding: one (batch, head) pair per core (B=2 x H=4 = 8 cores). Each core
computes its head's full pipeline and emits a partial [N, D] output (bf16);
the host sums the 4 head partials per batch in f32.

v3 design:
  * All heavy matmuls in bf16 (1 PE cycle/row vs 4 for fp32); PSUM accums f32.
  * Per-chunk LOCAL decay (no global cumsum carry chain): within chunk c,
    b = L^T g'' (local inclusive cumsum). q~=q*exp(-b/16), k~=k*exp(+b/16);
    cross-chunk state rescaled once per chunk by the per-feature factor
    f = exp(-b_last/16):  W_c = diag(f) (W_{c-1} + k~^T v), applied for free
    via the per-partition `scale=` AP of the PSUM->SBUF state eviction.
    Local exponent args <= ~6, safe in bf16/f32.
  * z-projection folded into the main projection blob (one pass over x per
    chunk); softplus runs per chunk on ACT/DVE.
  * ACT table discipline: exp+ln both resolve to the combined
    natural_log_exp_and_others table (the chooser is steered by blanking the
    exp-only/ln-only sets in the table list passed to the load-insertion
    pass; the chosen ids are real act_info.json sets, so hardware semantics
    are unchanged). Silu is computed as 0.5u(1+tanh(u/2)) in the final phase
    (one tanh-table load). ~2-3 table loads total (vs 33 in the baseline).
  * RMS r = (mean o^2 + eps)^-1/2 deferred: ssq accumulates per chunk, r is
    computed once (batched ln+exp) and folded into the silu gate.
  * Big contiguous DMAs (2KB rows); bf16 I/O, host sums partials in f32.
"""

import os
from contextlib import ExitStack

import numpy as np
import ml_dtypes

import concourse.bass as bass
import concourse.tile as tile
from concourse import bacc, mybir
from concourse.tile_rust import add_dep_helper
from concourse.bass_utils import run_bass_kernel_spmd

F32 = mybir.dt.float32
BF16 = mybir.dt.bfloat16
AF = mybir.ActivationFunctionType
ALU = mybir.AluOpType

B, N, D, H = 2, 1024, 1024, 4
KD, VD, DK, DV = 512, 1024, 128, 256
C = 128                    # chunk length (= token partitions)
NCH = N // C               # 8 chunks
NK = D // 128              # 8 contraction tiles
BLOBW = 896                # blob cols: q128 | k128 | v256 | gate256 | z128
GLN = 16.0
EPS = 1e-5

# module-level stash so test.py can grab profiling results
LAST_RESULTS = None

_BLANK_TABLES = ("exp_and_others", "natural_log", "exp_and_friends")
_tables_patched = False


def _patch_act_tables():
    """Steer the ACT-table-load chooser toward natural_log_exp_and_others so
    exp+ln never alternate table loads. Only the (name -> funcs) map used by
    the load-insertion pass and CoreSim is filtered; emitted act_func_set_ids
    still index the real act_info.json, so walrus/hardware see valid sets."""
    global _tables_patched
    if _tables_patched:
        return
    _tables_patched = True
    from concourse import hw_specs, bass_interp
    orig = hw_specs.get_activation_tables

    def patched(arch):
        tabs = dict(orig(arch))
        for name in _BLANK_TABLES:
            if name in tabs:
                tabs[name] = set()
        return tabs

    bacc.get_activation_tables = patched
    bass_interp.get_activation_tables = patched


def _emit_kernel(ctx: ExitStack, tc: "tile.TileContext", ap: dict):
    nc = tc.nc

    # Chain all PE instructions in program order (PE executes in-order; this
    # keeps the Tile scheduler from reordering matmuls within a PSUM bank,
    # which would break has_written clear ordering).
    pe_prev = [None]

    def mm(*args, **kw):
        inst = nc.tensor.matmul(*args, **kw)
        if pe_prev[0] is not None:
            add_dep_helper(inst.ins, pe_prev[0], sync=False, reason="pe-order")
        pe_prev[0] = inst.ins
        return inst

    def tr_(out, in_, ident):
        inst = nc.tensor.transpose(out, in_, ident)
        if pe_prev[0] is not None:
            add_dep_helper(inst.ins, pe_prev[0], sync=False, reason="pe-order")
        pe_prev[0] = inst.ins
        return inst

    xT, wblob, woutT = ap["xT"], ap["wblob"], ap["woutT"]
    bgk2, lmask, ident32, identb = ap["bgk2"], ap["lmask"], ap["ident32"], ap["identb"]
    out = ap["out"]

    consts = ctx.enter_context(tc.tile_pool(name="consts", bufs=1))
    wpool = ctx.enter_context(tc.tile_pool(name="wpool", bufs=1))
    work = ctx.enter_context(tc.tile_pool(name="work", bufs=2))
    wst = ctx.enter_context(tc.tile_pool(name="wst", bufs=2))
    store = ctx.enter_context(tc.tile_pool(name="store", bufs=1))
    outp = ctx.enter_context(tc.tile_pool(name="outp", bufs=2))
    ppool = ctx.enter_context(tc.tile_pool(name="ppool", bufs=2, space="PSUM"))
    ptr = ctx.enter_context(tc.tile_pool(name="ptr", bufs=2, space="PSUM"))
    pao = ctx.enter_context(tc.tile_pool(name="pao", bufs=1, space="PSUM"))
    pst = ctx.enter_context(tc.tile_pool(name="pst", bufs=1, space="PSUM"))

    # ---- constants ----
    L_sb = consts.tile([128, 128], F32)          # L[s,t]=1 iff s<=t (triu)
    nc.sync.dma_start(out=L_sb[:], in_=lmask[:])
    id_sb = consts.tile([128, 128], F32)
    nc.sync.dma_start(out=id_sb[:], in_=ident32[:])
    idb_sb = consts.tile([128, 128], BF16)
    nc.sync.dma_start(out=idb_sb[:], in_=identb[:])
    bg_sb = consts.tile([1, 128], BF16)
    nc.sync.dma_start(out=bg_sb[:], in_=bgk2[:])
    ones_row = consts.tile([1, 128], BF16)
    nc.vector.memset(ones_row[:], 1.0)
    ones_col = consts.tile([128, 1], BF16)
    nc.vector.memset(ones_col[:], 1.0)
    eps_sb = consts.tile([128, 1], F32)
    nc.vector.memset(eps_sb[:], EPS)

    # ---- weights + x (bf16), big contiguous transfers (2KB rows) ----
    xsb = wpool.tile([128, NK, N], BF16)
    for k in range(NK):
        nc.sync.dma_start(out=xsb[:, k, :], in_=xT[k])
    wb_sb = wpool.tile([128, NK, BLOBW], BF16)
    for k in range(NK):
        nc.sync.dma_start(out=wb_sb[:, k, :], in_=wblob[k])
    wout_sb = wpool.tile([128, 2, D], BF16)
    for j in range(2):
        nc.sync.dma_start(out=wout_sb[:, j, :], in_=woutT[j])

    # ---- main loop ---------------------------------------------------------
    # proj psum [128,1024]: bank0 {q 0:128 | k 128:256 | v 256:512}
    # bank1 {gate 512:768 | z 768:896 | b_loc 896:1024}. The bias matmul
    # closes bank1 so softplus can read z; the L-matmul (emitted later, after
    # the previous chunk's smalls, to give softplus time) writes b_loc into
    # the start-cleared region via skip_group_check, as in the carry pattern.
    def emit_proj(c):
        proj = ppool.tile([128, 1024], F32, tag="proj")
        tok = slice(c * C, (c + 1) * C)
        for k in range(NK):
            lhs = xsb[:, k, tok]
            mm(proj[:, 0:512], lhsT=lhs, rhs=wb_sb[:, k, 0:512],
               start=(k == 0), stop=(k == NK - 1))
            mm(proj[:, 512:896], lhsT=lhs, rhs=wb_sb[:, k, 512:896],
               start=(k == 0), stop=False)
        bias_mm = mm(proj[:, 768:896], lhsT=ones_row[:], rhs=bg_sb[:],
                     start=False, stop=True)
        # softplus: g = min(ln(1+exp(-z)), 48), per chunk (ACT/DVE only)
        e1 = work.tile([128, 128], F32, tag="e1")
        i = nc.scalar.activation(e1[:], proj[:, 768:896], AF.Exp, scale=-1.0)
        add_dep_helper(i.ins, bias_mm.ins, sync=False, reason="z after close")
        u1 = work.tile([128, 128], F32, tag="u1")
        nc.vector.tensor_scalar_add(u1[:], e1[:], 1.0)
        spt = work.tile([128, 128], F32, tag="sp")
        nc.scalar.activation(spt[:], u1[:], AF.Ln)
        g_c = work.tile([128, 128], F32, tag="g")
        nc.vector.tensor_scalar_min(g_c[:], spt[:], 48.0)
        return proj, g_c

    def emit_lmm(proj, g_c):
        return mm(proj[:, 896:1024], lhsT=L_sb[:], rhs=g_c[:],
                  start=False, stop=False, skip_group_check=True)

    state = {"w_prev": None}

    def emit_smalls(c, proj, lmm):
        # evictions + decay factors
        b_sb = work.tile([128, 128], F32, tag="b")
        i = nc.scalar.copy(b_sb[:], proj[:, 896:1024])
        add_dep_helper(i.ins, lmm.ins, sync=False, reason="b after L-mm")
        tr = ptr.tile([128, 512], F32, tag="tr")
        tr_(tr[:, 0:128], b_sb[:], id_sb[:])          # bT [feat, tok]
        En_tok = work.tile([128, 128], F32, tag="Ent")
        nc.scalar.activation(En_tok[:], b_sb[:], AF.Exp, scale=1.0 / GLN)
        ET = work.tile([128, 128], F32, tag="ET")
        nc.scalar.activation(ET[:], tr[:, 0:128], AF.Exp, scale=-1.0 / GLN)
        EnT = work.tile([128, 128], F32, tag="EnT")
        nc.scalar.activation(EnT[:], tr[:, 0:128], AF.Exp, scale=1.0 / GLN)
        f_vec = work.tile([128, 1], F32, tag="f")     # exp(-b_last/16) per feat
        nc.scalar.activation(f_vec[:], tr[:, 127:128], AF.Exp, scale=-1.0 / GLN)

        q_sb = work.tile([128, 128], F32, tag="q")
        nc.vector.tensor_copy(q_sb[:], proj[:, 0:128])
        tr_(tr[:, 128:256], q_sb[:], id_sb[:])
        qtT = work.tile([128, 128], BF16, tag="qtT")
        nc.vector.tensor_mul(qtT[:], tr[:, 128:256], ET[:])

        k_sb = work.tile([128, 128], F32, tag="k")
        nc.vector.tensor_copy(k_sb[:], proj[:, 128:256])
        tr_(tr[:, 256:384], k_sb[:], id_sb[:])
        ktT = work.tile([128, 128], BF16, tag="ktT")
        nc.vector.tensor_mul(ktT[:], tr[:, 256:384], EnT[:])
        kt_tm = work.tile([128, 128], BF16, tag="kt")
        nc.vector.tensor_mul(kt_tm[:], k_sb[:], En_tok[:])

        v_tm = work.tile([128, DV], BF16, tag="v")
        nc.scalar.copy(v_tm[:], proj[:, 256:512])
        ug = store.tile([128, DV], F32, tag=f"ug{c}")
        nc.scalar.copy(ug[:], proj[:, 512:768])

        # intra-chunk attention: AT[s,t] masked s<=t
        ao = pao.tile([128, 512], F32, tag="ao")      # at 0:128|oT 128:384|ssq
        mm(ao[:, 0:128], lhsT=ktT[:], rhs=qtT[:], start=True, stop=True)
        at_m = work.tile([128, 128], BF16, tag="atm")
        nc.vector.tensor_mul(at_m[:], ao[:, 0:128], L_sb[:])

        # oT = W_prev^T q~^T + v^T AT  (two dv halves)
        w_prev = state["w_prev"]
        if c > 0:
            mm(ao[:, 128:256], lhsT=w_prev[:, 0:128], rhs=qtT[:],
               start=False, stop=False, skip_group_check=True)
            mm(ao[:, 256:384], lhsT=w_prev[:, 128:256], rhs=qtT[:],
               start=False, stop=False, skip_group_check=True)
        mm(ao[:, 128:256], lhsT=v_tm[:, 0:128], rhs=at_m[:],
           start=False, stop=False, skip_group_check=True)
        mm(ao[:, 256:384], lhsT=v_tm[:, 128:256], rhs=at_m[:],
           start=False, stop=False, skip_group_check=True)

        # state: W_c = diag(f) (W_{c-1} + k~^T v)
        st = pst.tile([128, DV], F32, tag="st")
        mm(st[:], lhsT=kt_tm[:], rhs=v_tm[:], start=True, stop=(c == 0))
        if c > 0:
            mm(st[:], lhsT=idb_sb[:], rhs=w_prev[:], start=False, stop=True)
        w_new = wst.tile([128, DV], BF16, tag="w")
        nc.scalar.activation(w_new[:], st[:], AF.Copy, scale=f_vec[:])
        state["w_prev"] = w_new

        # ssq per token -> spare column of the at/ot bank, then to SBUF
        sq = work.tile([128, DV], BF16, tag="sq")
        nc.scalar.square(sq[:], ao[:, 128:384])
        mm(ao[:, 384:385], lhsT=sq[:, 0:128], rhs=ones_col[:],
           start=False, stop=False, skip_group_check=True)
        mm(ao[:, 384:385], lhsT=sq[:, 128:256], rhs=ones_col[:],
           start=False, stop=False, skip_group_check=True)
        nc.vector.tensor_copy(ssq_all[:, c:c + 1], ao[:, 384:385])

        oT = store.tile([128, DV], BF16, tag=f"oT{c}")
        nc.vector.tensor_copy(oT[:], ao[:, 128:384])
        state[f"oT{c}"] = oT
        state[f"ug{c}"] = ug

    ssq_all = wpool.tile([128, 8], F32)

    # software pipeline: proj(c+1) before smalls(c); L-mm(c+1) after smalls(c)
    # so the per-chunk softplus (ACT/DVE) overlaps chunk c's small PE ops.
    proj0, g0 = emit_proj(0)
    lmm0 = emit_lmm(proj0, g0)
    cur = (proj0, lmm0)
    nxt = None
    for c in range(NCH):
        if c + 1 < NCH:
            pj, gc = emit_proj(c + 1)
        emit_smalls(c, cur[0], cur[1])
        if c + 1 < NCH:
            lm = emit_lmm(pj, gc)
            cur = (pj, lm)

    # ---- Phase D: RMS scale, silu gate (via tanh), final projection --------
    s_sb = work.tile([128, 8], F32, tag="s")
    nc.scalar.activation(s_sb[:], ssq_all[:], AF.Ln, scale=1.0 / DV,
                         bias=eps_sb[:])
    r_all = work.tile([128, 8], F32, tag="r")
    nc.scalar.activation(r_all[:], s_sb[:], AF.Exp, scale=-0.5)

    for c in range(NCH):
        tok = slice(c * C, (c + 1) * C)
        ug = state[f"ug{c}"]
        oT = state[f"oT{c}"]
        th = work.tile([128, DV], F32, tag="th")
        nc.scalar.activation(th[:], ug[:], AF.Tanh, scale=0.5)
        thp = work.tile([128, DV], F32, tag="thp")
        nc.vector.tensor_scalar(thp[:], th[:], 0.5, 0.5, ALU.mult, ALU.add)
        # gate = silu(ug) * r = (ug*r) * (0.5 + 0.5*tanh(ug/2))
        gate_tm = work.tile([128, DV], F32, tag="gate")
        nc.vector.scalar_tensor_tensor(gate_tm[:], ug[:], r_all[:, c:c + 1],
                                       thp[:], ALU.mult, ALU.mult)
        tr2 = ptr.tile([128, 512], F32, tag="tr")
        tr_(tr2[:, 0:128], gate_tm[:, 0:128], id_sb[:])
        tr_(tr2[:, 128:256], gate_tm[:, 128:256], id_sb[:])
        gateT = work.tile([128, DV], BF16, tag="gT")
        nc.scalar.copy(gateT[:], tr2[:, 0:256])
        og = work.tile([128, DV], BF16, tag="og")
        nc.vector.tensor_mul(og[:], oT[:], gateT[:])

        fin = ppool.tile([128, 1024], F32, tag="proj")
        for nb in range(2):
            cols = slice(nb * 512, (nb + 1) * 512)
            mm(fin[:, cols], lhsT=og[:, 0:128],
               rhs=wout_sb[:, 0, cols], start=True, stop=False)
            mm(fin[:, cols], lhsT=og[:, 128:256],
               rhs=wout_sb[:, 1, cols], start=False, stop=True)
        o_sb = outp.tile([128, 1024], BF16, tag="o")
        nc.scalar.copy(o_sb[:, 0:512], fin[:, 0:512])
        nc.vector.tensor_copy(o_sb[:, 512:1024], fin[:, 512:1024])
        nc.sync.dma_start(out=out[tok, :], in_=o_sb[:])


def _build_nc():
    _patch_act_tables()
    nc = bacc.Bacc("TRN2", target_bir_lowering=False, debug=False, num_devices=8)
    ap = {
        "xT": nc.dram_tensor("xT", [NK, 128, N], BF16, kind="ExternalInput").ap(),
        "wblob": nc.dram_tensor("wblob", [NK, 128, BLOBW], BF16,
                                kind="ExternalInput").ap(),
        "woutT": nc.dram_tensor("woutT", [2, 128, D], BF16,
                                kind="ExternalInput").ap(),
        "bgk2": nc.dram_tensor("bgk2", [1, 128], BF16, kind="ExternalInput").ap(),
        "lmask": nc.dram_tensor("lmask", [128, 128], F32,
                                kind="ExternalInput").ap(),
        "ident32": nc.dram_tensor("ident32", [128, 128], F32,
                                  kind="ExternalInput").ap(),
        "identb": nc.dram_tensor("identb", [128, 128], BF16,
                                 kind="ExternalInput").ap(),
        "out": nc.dram_tensor("out", [N, D], BF16, kind="ExternalOutput").ap(),
    }
    with tile.TileContext(nc) as tc:
        with ExitStack() as ctx:
            _emit_kernel(ctx, tc, ap)
    nc.compile()
    return nc


def kernel(x, Wq, Wk, Wv, Wg, Wgk1, Wgk2, bgk2, Wout, rms_w):
    global LAST_RESULTS
    BF = ml_dtypes.bfloat16
    x = np.asarray(x, np.float32)
    Wz = (np.asarray(Wgk1, np.float32) @ np.asarray(Wgk2, np.float32))
    L = np.triu(np.ones((C, C), np.float32))
    I32 = np.eye(128, dtype=np.float32)
    Ib = np.eye(128, dtype=np.float32).astype(BF)

    in_maps = []
    for core in range(8):
        b, h = core // H, core % H
        xTb = np.ascontiguousarray(x[b].T).reshape(NK, 128, N).astype(BF)
        blob = np.ascontiguousarray(np.concatenate([
            Wq[:, h * DK:(h + 1) * DK], Wk[:, h * DK:(h + 1) * DK],
            Wv[:, h * DV:(h + 1) * DV], Wg[:, h * DV:(h + 1) * DV],
            Wz[:, h * DK:(h + 1) * DK]],
            axis=1).astype(np.float32)).reshape(NK, 128, BLOBW).astype(BF)
        woutP = np.ascontiguousarray(
            (np.asarray(rms_w, np.float32)[:, None]
             * np.asarray(Wout, np.float32)[h * DV:(h + 1) * DV])
        ).reshape(2, 128, D).astype(BF)
        in_maps.append({
            "xT": xTb,
            "wblob": blob,
            "woutT": woutP,
            "bgk2": np.ascontiguousarray(
                np.asarray(bgk2, np.float32)[h * DK:(h + 1) * DK][None, :]
            ).astype(BF),
            "lmask": L,
            "ident32": I32,
            "identb": Ib,
        })

    nc = _build_nc()
    trace = os.environ.get("BASSGLA_TRACE", "0") == "1"
    res = run_bass_kernel_spmd(nc, in_maps, list(range(8)), trace=trace)
    LAST_RESULTS = res

    out = np.zeros((B, N, D), np.float32)
    for core in range(8):
        out[core // H] += np.asarray(res.results[core]["out"], np.float32)
    return out


# revision 7
# speedup vs baseline: 2.2519x; 1.0216x over previous
"""Gated Linear Attention on 8 Trainium2 NeuronCores.

Sharding: one (batch, head) pair per core (B=2 x H=4 = 8 cores). Each core
computes its head's full pipeline and emits a partial [N, D] output (bf16);
the host sums the 4 head partials per batch in f32.

v3 design:
  * All heavy matmuls in bf16 (1 PE cycle/row vs 4 for fp32); PSUM accums f32.
  * Per-chunk LOCAL decay (no global cumsum carry chain): within chunk c,
    b = L^T g'' (local inclusive cumsum). q~=q*exp(-b/16), k~=k*exp(+b/16);
    cross-chunk state rescaled once per chunk by the per-feature factor
    f = exp(-b_last/16):  W_c = diag(f) (W_{c-1} + k~^T v), applied for free
    via the per-partition `scale=` AP of the PSUM->SBUF state eviction.
    Local exponent args <= ~6, safe in bf16/f32.
  * z-projection folded into the main projection blob (one pass over x per
    chunk); softplus runs per chunk on ACT/DVE.
  * ACT table discipline: exp+ln both resolve to the combined
    natural_log_exp_and_others table (the chooser is steered by blanking the
    exp-only/ln-only sets in the table list passed to the load-insertion
    pass; the chosen ids are real act_info.json sets, so hardware semantics
    are unchanged). Silu is computed as 0.5u(1+tanh(u/2)) in the final phase
    (one tanh-table load). ~2-3 table loads total (vs 33 in the baseline).
  * RMS r = (mean o^2 + eps)^-1/2 deferred: ssq accumulates per chunk, r is
    computed once (batched ln+exp) and folded into the silu gate.
  * Big contiguous DMAs (2KB rows); bf16 I/O, host sums partials in f32.
"""

import os
from contextlib import ExitStack

import numpy as np
import ml_dtypes

import concourse.bass as bass
import concourse.tile as tile
from concourse import bacc, mybir
from concourse.tile_rust import add_dep_helper
from concourse.bass_utils import run_bass_kernel_spmd

F32 = mybir.dt.float32
BF16 = mybir.dt.bfloat16
AF = mybir.ActivationFunctionType
ALU = mybir.AluOpType

B, N, D, H = 2, 1024, 1024, 4
KD, VD, DK, DV = 512, 1024, 128, 256
C = 128                    # chunk length (= token partitions)
NCH = N // C               # 8 chunks
NK = D // 128              # 8 contraction tiles
BLOBW = 896                # blob cols: q128 | k128 | v256 | gate256 | z128
GLN = 16.0
EPS = 1e-5

# module-level stash so test.py can grab profiling results
LAST_RESULTS = None

_BLANK_TABLES = ("exp_and_others", "natural_log", "exp_and_friends")
_tables_patched = False


def _patch_act_tables():
    """Steer the ACT-table-load chooser toward natural_log_exp_and_others so
    exp+ln never alternate table loads. Only the (name -> funcs) map used by
    the load-insertion pass and CoreSim is filtered; emitted act_func_set_ids
    still index the real act_info.json, so walrus/hardware see valid sets."""
    global _tables_patched
    if _tables_patched:
        return
    _tables_patched = True
    from concourse import hw_specs, bass_interp
    orig = hw_specs.get_activation_tables

    def patched(arch):
        tabs = dict(orig(arch))
        for name in _BLANK_TABLES:
            if name in tabs:
                tabs[name] = set()
        return tabs

    bacc.get_activation_tables = patched
    bass_interp.get_activation_tables = patched


def _emit_kernel(ctx: ExitStack, tc: "tile.TileContext", ap: dict):
    nc = tc.nc

    # Chain all PE instructions in program order (PE executes in-order; this
    # keeps the Tile scheduler from reordering matmuls within a PSUM bank,
    # which would break has_written clear ordering).
    pe_prev = [None]

    def mm(*args, **kw):
        inst = nc.tensor.matmul(*args, **kw)
        if pe_prev[0] is not None:
            add_dep_helper(inst.ins, pe_prev[0], sync=False, reason="pe-order")
        pe_prev[0] = inst.ins
        return inst

    def tr_(out, in_, ident):
        inst = nc.tensor.transpose(out, in_, ident)
        if pe_prev[0] is not None:
            add_dep_helper(inst.ins, pe_prev[0], sync=False, reason="pe-order")
        pe_prev[0] = inst.ins
        return inst

    xT, wblob, woutT = ap["xT"], ap["wblob"], ap["woutT"]
    bgk2, lmask, ident32, identb = ap["bgk2"], ap["lmask"], ap["ident32"], ap["identb"]
    out = ap["out"]

    consts = ctx.enter_context(tc.tile_pool(name="consts", bufs=1))
    wpool = ctx.enter_context(tc.tile_pool(name="wpool", bufs=1))
    work = ctx.enter_context(tc.tile_pool(name="work", bufs=2))
    wst = ctx.enter_context(tc.tile_pool(name="wst", bufs=2))
    store = ctx.enter_context(tc.tile_pool(name="store", bufs=1))
    outp = ctx.enter_context(tc.tile_pool(name="outp", bufs=2))
    ppool = ctx.enter_context(tc.tile_pool(name="ppool", bufs=2, space="PSUM"))
    ptr = ctx.enter_context(tc.tile_pool(name="ptr", bufs=2, space="PSUM"))
    pao = ctx.enter_context(tc.tile_pool(name="pao", bufs=1, space="PSUM"))
    pst = ctx.enter_context(tc.tile_pool(name="pst", bufs=1, space="PSUM"))

    # ---- constants ----
    L_sb = consts.tile([128, 128], F32)          # L[s,t]=1 iff s<=t (triu)
    nc.sync.dma_start(out=L_sb[:], in_=lmask[:])
    id_sb = consts.tile([128, 128], F32)
    nc.sync.dma_start(out=id_sb[:], in_=ident32[:])
    idb_sb = consts.tile([128, 128], BF16)
    nc.sync.dma_start(out=idb_sb[:], in_=identb[:])
    bg_sb = consts.tile([1, 128], BF16)
    nc.sync.dma_start(out=bg_sb[:], in_=bgk2[:])
    ones_row = consts.tile([1, 128], BF16)
    nc.vector.memset(ones_row[:], 1.0)
    ones_col = consts.tile([128, 1], BF16)
    nc.vector.memset(ones_col[:], 1.0)
    eps_sb = consts.tile([128, 1], F32)
    nc.vector.memset(eps_sb[:], EPS)

    # ---- weights + x (bf16), big contiguous transfers (2KB rows) ----
    # x DRAM layout [128, N, NK]: one 256KB DMA per token chunk, so chunk-0
    # compute starts ~1.5us in instead of after the full 2MB.
    xsb = wpool.tile([128, N, NK], BF16)
    nc.sync.dma_start(out=xsb[:, 0:C, :], in_=xT[:, 0:C, :])
    wb_sb = wpool.tile([128, NK, BLOBW], BF16)
    for k in range(NK):
        nc.sync.dma_start(out=wb_sb[:, k, :], in_=wblob[k])
    for c in range(1, NCH):
        nc.sync.dma_start(out=xsb[:, c * C:(c + 1) * C, :],
                          in_=xT[:, c * C:(c + 1) * C, :])
    wout_sb = wpool.tile([128, 2, D], BF16)
    for j in range(2):
        nc.sync.dma_start(out=wout_sb[:, j, :], in_=woutT[j])

    # ---- main loop ---------------------------------------------------------
    # proj psum [128,1024]: bank0 {q 0:128 | k 128:256 | v 256:512}
    # bank1 {gate 512:768 | z 768:896 | b_loc 896:1024}. The bias matmul
    # closes bank1 so softplus can read z; the L-matmul (emitted later, after
    # the previous chunk's smalls, to give softplus time) writes b_loc into
    # the start-cleared region via skip_group_check, as in the carry pattern.
    def emit_proj(c):
        proj = ppool.tile([128, 1024], F32, tag="proj")
        tok = slice(c * C, (c + 1) * C)
        for k in range(NK):
            lhs = xsb[:, tok, k]
            mm(proj[:, 0:512], lhsT=lhs, rhs=wb_sb[:, k, 0:512],
               start=(k == 0), stop=(k == NK - 1))
            mm(proj[:, 512:896], lhsT=lhs, rhs=wb_sb[:, k, 512:896],
               start=(k == 0), stop=False)
        bias_mm = mm(proj[:, 768:896], lhsT=ones_row[:], rhs=bg_sb[:],
                     start=False, stop=True)
        # softplus: g = min(ln(1+exp(-z)), 48), per chunk (ACT/DVE only)
        e1 = work.tile([128, 128], F32, tag="e1")
        i = nc.scalar.activation(e1[:], proj[:, 768:896], AF.Exp, scale=-1.0)
        add_dep_helper(i.ins, bias_mm.ins, sync=False, reason="z after close")
        u1 = work.tile([128, 128], F32, tag="u1")
        nc.vector.tensor_scalar_add(u1[:], e1[:], 1.0)
        spt = work.tile([128, 128], F32, tag="sp")
        nc.scalar.activation(spt[:], u1[:], AF.Ln)
        g_c = work.tile([128, 128], F32, tag="g")
        nc.vector.tensor_scalar_min(g_c[:], spt[:], 48.0)
        return proj, g_c

    def emit_lmm(proj, g_c):
        return mm(proj[:, 896:1024], lhsT=L_sb[:], rhs=g_c[:],
                  start=False, stop=False, skip_group_check=True)

    state = {"w_prev": None}

    def emit_smalls(c, proj, lmm):
        # evictions + decay factors
        b_sb = work.tile([128, 128], F32, tag="b")
        i = nc.scalar.copy(b_sb[:], proj[:, 896:1024])
        add_dep_helper(i.ins, lmm.ins, sync=False, reason="b after L-mm")
        tr = ptr.tile([128, 512], F32, tag="tr")
        tr_(tr[:, 0:128], b_sb[:], id_sb[:])          # bT [feat, tok]
        En_tok = work.tile([128, 128], F32, tag="Ent")
        nc.scalar.activation(En_tok[:], b_sb[:], AF.Exp, scale=1.0 / GLN)
        ET = work.tile([128, 128], F32, tag="ET")
        nc.scalar.activation(ET[:], tr[:, 0:128], AF.Exp, scale=-1.0 / GLN)
        EnT = work.tile([128, 128], F32, tag="EnT")
        nc.scalar.activation(EnT[:], tr[:, 0:128], AF.Exp, scale=1.0 / GLN)
        f_vec = work.tile([128, 1], F32, tag="f")     # exp(-b_last/16) per feat
        nc.scalar.activation(f_vec[:], tr[:, 127:128], AF.Exp, scale=-1.0 / GLN)

        q_sb = work.tile([128, 128], F32, tag="q")
        nc.vector.tensor_copy(q_sb[:], proj[:, 0:128])
        tr_(tr[:, 128:256], q_sb[:], id_sb[:])
        qtT = work.tile([128, 128], BF16, tag="qtT")
        nc.vector.tensor_mul(qtT[:], tr[:, 128:256], ET[:])

        k_sb = work.tile([128, 128], F32, tag="k")
        nc.vector.tensor_copy(k_sb[:], proj[:, 128:256])
        tr_(tr[:, 256:384], k_sb[:], id_sb[:])
        ktT = work.tile([128, 128], BF16, tag="ktT")
        nc.vector.tensor_mul(ktT[:], tr[:, 256:384], EnT[:])
        kt_tm = work.tile([128, 128], BF16, tag="kt")
        nc.vector.tensor_mul(kt_tm[:], k_sb[:], En_tok[:])

        v_tm = work.tile([128, DV], BF16, tag="v")
        nc.scalar.copy(v_tm[:], proj[:, 256:512])
        ug = store.tile([128, DV], F32, tag=f"ug{c}")
        nc.scalar.copy(ug[:], proj[:, 512:768])

        # intra-chunk attention: AT[s,t] masked s<=t
        ao = pao.tile([128, 512], F32, tag="ao")      # at 0:128|oT 128:384|ssq
        mm(ao[:, 0:128], lhsT=ktT[:], rhs=qtT[:], start=True, stop=True)
        at_m = work.tile([128, 128], BF16, tag="atm")
        nc.vector.tensor_mul(at_m[:], ao[:, 0:128], L_sb[:])

        # oT = W_prev^T q~^T + v^T AT  (two dv halves)
        w_prev = state["w_prev"]
        if c > 0:
            mm(ao[:, 128:256], lhsT=w_prev[:, 0:128], rhs=qtT[:],
               start=False, stop=False, skip_group_check=True)
            mm(ao[:, 256:384], lhsT=w_prev[:, 128:256], rhs=qtT[:],
               start=False, stop=False, skip_group_check=True)
        mm(ao[:, 128:256], lhsT=v_tm[:, 0:128], rhs=at_m[:],
           start=False, stop=False, skip_group_check=True)
        mm(ao[:, 256:384], lhsT=v_tm[:, 128:256], rhs=at_m[:],
           start=False, stop=False, skip_group_check=True)

        # state: W_c = diag(f) (W_{c-1} + k~^T v)
        st = pst.tile([128, DV], F32, tag="st")
        mm(st[:], lhsT=kt_tm[:], rhs=v_tm[:], start=True, stop=(c == 0))
        if c > 0:
            mm(st[:], lhsT=idb_sb[:], rhs=w_prev[:], start=False, stop=True)
        w_new = wst.tile([128, DV], BF16, tag="w")
        nc.scalar.activation(w_new[:], st[:], AF.Copy, scale=f_vec[:])
        state["w_prev"] = w_new

        # ssq per token -> spare column of the at/ot bank, then to SBUF
        sq = work.tile([128, DV], BF16, tag="sq")
        nc.scalar.square(sq[:], ao[:, 128:384])
        mm(ao[:, 384:385], lhsT=sq[:, 0:128], rhs=ones_col[:],
           start=False, stop=False, skip_group_check=True)
        mm(ao[:, 384:385], lhsT=sq[:, 128:256], rhs=ones_col[:],
           start=False, stop=False, skip_group_check=True)
        nc.vector.tensor_copy(ssq_all[:, c:c + 1], ao[:, 384:385])

        oT = store.tile([128, DV], BF16, tag=f"oT{c}")
        nc.vector.tensor_copy(oT[:], ao[:, 128:384])
        state[f"oT{c}"] = oT
        state[f"ug{c}"] = ug

    ssq_all = wpool.tile([128, 8], F32)

    # software pipeline: proj(c+1) before smalls(c); L-mm(c+1) after smalls(c)
    # so the per-chunk softplus (ACT/DVE) overlaps chunk c's small PE ops.
    proj0, g0 = emit_proj(0)
    lmm0 = emit_lmm(proj0, g0)
    cur = (proj0, lmm0)
    nxt = None
    for c in range(NCH):
        if c + 1 < NCH:
            pj, gc = emit_proj(c + 1)
        emit_smalls(c, cur[0], cur[1])
        if c + 1 < NCH:
            lm = emit_lmm(pj, gc)
            cur = (pj, lm)

    # ---- Phase D: RMS scale, silu gate (via tanh), final projection --------
    s_sb = work.tile([128, 8], F32, tag="s")
    nc.scalar.activation(s_sb[:], ssq_all[:], AF.Ln, scale=1.0 / DV,
                         bias=eps_sb[:])
    r_all = work.tile([128, 8], F32, tag="r")
    r_ins = nc.scalar.activation(r_all[:], s_sb[:], AF.Exp, scale=-0.5)

    for c in range(NCH):
        tok = slice(c * C, (c + 1) * C)
        ug = state[f"ug{c}"]
        oT = state[f"oT{c}"]
        th = work.tile([128, DV], F32, tag="th")
        i = nc.scalar.activation(th[:], ug[:], AF.Tanh, scale=0.5)
        # keep all tanh after the main loop's exp/ln (one table switch)
        add_dep_helper(i.ins, r_ins.ins, sync=False, reason="tanh after r")
        thp = work.tile([128, DV], F32, tag="thp")
        nc.vector.tensor_scalar(thp[:], th[:], 0.5, 0.5, ALU.mult, ALU.add)
        # gate = silu(ug) * r = (ug*r) * (0.5 + 0.5*tanh(ug/2))
        gate_tm = work.tile([128, DV], F32, tag="gate")
        nc.vector.scalar_tensor_tensor(gate_tm[:], ug[:], r_all[:, c:c + 1],
                                       thp[:], ALU.mult, ALU.mult)
        tr2 = ptr.tile([128, 512], F32, tag="tr")
        tr_(tr2[:, 0:128], gate_tm[:, 0:128], id_sb[:])
        tr_(tr2[:, 128:256], gate_tm[:, 128:256], id_sb[:])
        gateT = work.tile([128, DV], BF16, tag="gT")
        nc.scalar.copy(gateT[:], tr2[:, 0:256])
        og = work.tile([128, DV], BF16, tag="og")
        nc.vector.tensor_mul(og[:], oT[:], gateT[:])

        fin = ppool.tile([128, 1024], F32, tag="proj")
        for nb in range(2):
            cols = slice(nb * 512, (nb + 1) * 512)
            mm(fin[:, cols], lhsT=og[:, 0:128],
               rhs=wout_sb[:, 0, cols], start=True, stop=False)
            mm(fin[:, cols], lhsT=og[:, 128:256],
               rhs=wout_sb[:, 1, cols], start=False, stop=True)
        o_sb = outp.tile([128, 1024], BF16, tag="o")
        nc.scalar.copy(o_sb[:, 0:512], fin[:, 0:512])
        nc.vector.tensor_copy(o_sb[:, 512:1024], fin[:, 512:1024])
        nc.gpsimd.dma_start(out=out[tok, :], in_=o_sb[:])


def _build_nc():
    _patch_act_tables()
    nc = bacc.Bacc("TRN2", target_bir_lowering=False, debug=False, num_devices=8)
    ap = {
        "xT": nc.dram_tensor("xT", [128, N, NK], BF16, kind="ExternalInput").ap(),
        "wblob": nc.dram_tensor("wblob", [NK, 128, BLOBW], BF16,
                                kind="ExternalInput").ap(),
        "woutT": nc.dram_tensor("woutT", [2, 128, D], BF16,
                                kind="ExternalInput").ap(),
        "bgk2": nc.dram_tensor("bgk2", [1, 128], BF16, kind="ExternalInput").ap(),
        "lmask": nc.dram_tensor("lmask", [128, 128], F32,
                                kind="ExternalInput").ap(),
        "ident32": nc.dram_tensor("ident32", [128, 128], F32,
                                  kind="ExternalInput").ap(),
        "identb": nc.dram_tensor("identb", [128, 128], BF16,
                                 kind="ExternalInput").ap(),
        "out": nc.dram_tensor("out", [N, D], BF16, kind="ExternalOutput").ap(),
    }
    with tile.TileContext(nc) as tc:
        with ExitStack() as ctx:
            _emit_kernel(ctx, tc, ap)
    nc.compile()
    return nc


def kernel(x, Wq, Wk, Wv, Wg, Wgk1, Wgk2, bgk2, Wout, rms_w):
    global LAST_RESULTS
    BF = ml_dtypes.bfloat16
    x = np.asarray(x, np.float32)
    Wz = (np.asarray(Wgk1, np.float32) @ np.asarray(Wgk2, np.float32))
    L = np.triu(np.ones((C, C), np.float32))
    I32 = np.eye(128, dtype=np.float32)
    Ib = np.eye(128, dtype=np.float32).astype(BF)

    in_maps = []
    for core in range(8):
        b, h = core // H, core % H
        xTb = np.ascontiguousarray(
            x[b].T.reshape(NK, 128, N).transpose(1, 2, 0)).astype(BF)
        blob = np.ascontiguousarray(np.concatenate([
            Wq[:, h * DK:(h + 1) * DK], Wk[:, h * DK:(h + 1) * DK],
            Wv[:, h * DV:(h + 1) * DV], Wg[:, h * DV:(h + 1) * DV],
            Wz[:, h * DK:(h + 1) * DK]],
            axis=1).astype(np.float32)).reshape(NK, 128, BLOBW).astype(BF)
        woutP = np.ascontiguousarray(
            (np.asarray(rms_w, np.float32)[:, None]
             * np.asarray(Wout, np.float32)[h * DV:(h + 1) * DV])
        ).reshape(2, 128, D).astype(BF)
        in_maps.append({
            "xT": xTb,
            "wblob": blob,
            "woutT": woutP,
            "bgk2": np.ascontiguousarray(
                np.asarray(bgk2, np.float32)[h * DK:(h + 1) * DK][None, :]
            ).astype(BF),
            "lmask": L,
            "ident32": I32,
            "identb": Ib,
        })

    nc = _build_nc()
    trace = os.environ.get("BASSGLA_TRACE", "0") == "1"
    res = run_bass_kernel_spmd(nc, in_maps, list(range(8)), trace=trace)
    LAST_RESULTS = res

    out = np.zeros((B, N, D), np.float32)
    for core in range(8):
        out[core // H] += np.asarray(res.results[core]["out"], np.float32)
    return out
